# revision 7
# baseline (speedup 1.0000x reference)
"""BiMPM matching-layer kernel for Trainium2 (8 NeuronCores, pure data parallel).

v2: restructured for engine balance.
 - All matmuls bf16 except the m4-argmax chain (G, rrow, rrep) which must be
   fp32-exact so the argmax matches the reference (a flipped index gathers a
   completely different q0 row).
 - m2 (max-pooling): per perspective l, scale v2 by w2_l (rows) and 1/n2_l
   (cols) BEFORE the num matmul.  The [H,256] column-scale tile M_l =
   w2_l(h) * r2(l,j) is produced by one K=20 matmul against a host constant
   W2E whose l-th block is the one-hot row l scaled by w2_l.  The max over q
   runs hierarchically: Pool tensor_max halves PSUM (L1), DVE batched
   tensor_max (L2), DVE batched tensor_reduce tail.
Output (B, S1, 8*L), L=20. Batch 64 -> 8 per core.
"""
import numpy as np

B, S, H, L = 64, 256, 128, 20
NCORES = 8
BPC = B // NCORES
EPS = 1e-8

_cache = {}


def _build_bass():
    from contextlib import ExitStack

    import concourse.bass as bass
    import concourse.tile as tile
    from concourse import mybir

    f32 = mybir.dt.float32
    bf16 = mybir.dt.bfloat16
    AF = mybir.ActivationFunctionType
    OP = mybir.AluOpType
    AX = mybir.AxisListType

    nc = bass.Bass()

    # DRAM I/O (per core)
    pTb = nc.dram_tensor("pTb", [BPC, 2, H, S], bf16, kind="ExternalInput")
    qTb = nc.dram_tensor("qTb", [BPC, 2, H, S], bf16, kind="ExternalInput")
    pTf = nc.dram_tensor("pTf", [BPC, 2, H, S], f32, kind="ExternalInput")
    qTf = nc.dram_tensor("qTf", [BPC, 2, H, S], f32, kind="ExternalInput")
    qn = nc.dram_tensor("qn", [BPC, 2, S, H], f32, kind="ExternalInput")
    qnb = nc.dram_tensor("qnb", [BPC, 2, S, H], bf16, kind="ExternalInput")
    q0nb = nc.dram_tensor("q0nb", [2, S, H], bf16, kind="ExternalInput")
    w2T = nc.dram_tensor("w2T", [2, H, 81], f32, kind="ExternalInput")
    w2Tb = nc.dram_tensor("w2Tb", [2, H, 81], bf16, kind="ExternalInput")
    w2E = nc.dram_tensor("w2E", [2, L, L * H], bf16, kind="ExternalInput")
    iota2 = nc.dram_tensor("iota2", [H, 2], f32, kind="ExternalInput")
    onesr = nc.dram_tensor("onesr", [1, H], f32, kind="ExternalInput")
    onesrb = nc.dram_tensor("onesrb", [1, H], bf16, kind="ExternalInput")
    onesc = nc.dram_tensor("onesc", [H, 1], f32, kind="ExternalInput")
    ident = nc.dram_tensor("ident", [H, H], f32, kind="ExternalInput")
    out = nc.dram_tensor("out", [BPC, S, 8 * L], f32, kind="ExternalOutput")

    with tile.TileContext(nc) as tc, ExitStack() as ctx:
        cpool = ctx.enter_context(tc.tile_pool(name="consts", bufs=1))
        inp = ctx.enter_context(tc.tile_pool(name="inp", bufs=3))
        mid = ctx.enter_context(tc.tile_pool(name="mid", bufs=2))
        m2p = ctx.enter_context(tc.tile_pool(name="m2p", bufs=3))
        big = ctx.enter_context(tc.tile_pool(name="big", bufs=2))
        outp = ctx.enter_context(tc.tile_pool(name="outp", bufs=3))
        # PSUM (8 banks): each [H,512] f32 tile = 1 bank
        psQ_p = ctx.enter_context(tc.tile_pool(name="psQ", bufs=1, space="PSUM"))
        psG_p = ctx.enter_context(tc.tile_pool(name="psG", bufs=1, space="PSUM"))
        psWork_p = ctx.enter_context(tc.tile_pool(name="psWork", bufs=1, space="PSUM"))
        psSmall_p = ctx.enter_context(tc.tile_pool(name="psSmall", bufs=1, space="PSUM"))
        psM_p = ctx.enter_context(tc.tile_pool(name="psM", bufs=1, space="PSUM"))
        psNum_p = ctx.enter_context(tc.tile_pool(name="psNum", bufs=1, space="PSUM"))

        # ---- constants ----
        w2_sb = cpool.tile([H, 2, 81], f32)
        nc.sync.dma_start(out=w2_sb, in_=w2T[:].rearrange("d h c -> h d c"))
        w2b_sb = cpool.tile([H, 2, 81], bf16)
        nc.sync.dma_start(out=w2b_sb, in_=w2Tb[:].rearrange("d h c -> h d c"))
        w2e_sb = cpool.tile([L, 2, L * H], bf16)
        nc.sync.dma_start(out=w2e_sb, in_=w2E[:].rearrange("d l c -> l d c"))
        q0c = cpool.tile([H, 2, 2, H], bf16)
        nc.sync.dma_start(out=q0c, in_=q0nb[:].rearrange("d (c q) h -> q d c h", c=2))
        iota_sb = cpool.tile([H, 2], f32)
        nc.sync.dma_start(out=iota_sb, in_=iota2[:])
        iotab_sb = cpool.tile([H, 2], bf16)
        nc.vector.tensor_copy(iotab_sb, iota_sb)
        onesr_sb = cpool.tile([1, H], f32)
        nc.sync.dma_start(out=onesr_sb, in_=onesr[:])
        onesrb_sb = cpool.tile([1, H], bf16)
        nc.sync.dma_start(out=onesrb_sb, in_=onesrb[:])
        onesc_sb = cpool.tile([H, 1], f32)
        nc.sync.dma_start(out=onesc_sb, in_=onesc[:])
        ident_sb = cpool.tile([H, H], f32)
        nc.sync.dma_start(out=ident_sb, in_=ident[:])

        for b in range(BPC):
            outt0 = outp.tile([H, 8 * L], f32, name="outt0")
            outt1 = outp.tile([H, 8 * L], f32, name="outt1")
            outts = (outt0, outt1)
            for d in range(2):
                w2d = w2_sb[:, d, :]
                w2bd = w2b_sb[:, d, :]
                # ---- loads ----
                v1b = inp.tile([H, S], bf16, name="v1b")
                nc.sync.dma_start(out=v1b, in_=pTb[b, d])
                v2b = inp.tile([H, S], bf16, name="v2b")
                nc.sync.dma_start(out=v2b, in_=qTb[b, d])
                v1f = inp.tile([H, S], f32, name="v1f")
                nc.sync.dma_start(out=v1f, in_=pTf[b, d])
                v2f = inp.tile([H, S], f32, name="v2f")
                nc.sync.dma_start(out=v2f, in_=qTf[b, d])
                v2n = inp.tile([H, 2, H], f32, name="v2n")
                nc.sync.dma_start(out=v2n, in_=qn[b, d].rearrange("(c q) h -> q c h", c=2))
                v2nb = inp.tile([H, 2, H], bf16, name="v2nb")
                nc.sync.dma_start(out=v2nb, in_=qnb[b, d].rearrange("(c q) h -> q c h", c=2))

                # ---- squares (Pool: SBUF-only ops, keeps ACT free) ----
                v1sqb = mid.tile([H, S], bf16, name="v1sqb")
                nc.gpsimd.tensor_tensor(v1sqb, v1b, v1b, op=OP.mult)
                v2sqb = mid.tile([H, S], bf16, name="v2sqb")
                nc.gpsimd.tensor_tensor(v2sqb, v2b, v2b, op=OP.mult)
                v2sq = mid.tile([H, S], f32, name="v2sq")
                nc.scalar.square(v2sq, v2f)

                # ---- norms (one bank) ----
                # psQ: [rows 0:20 cols 0:256 n2m2T | row 32 cols 0:256 rrow |
                #       cols 256:418 n1 (2x81) | cols 418:420 ||v2_j||^2 fp32]
                psQ = psQ_p.tile([H, 512], f32, name="psQ")
                nc.tensor.matmul(psQ[0:L, 0:256], w2bd[:, 20:40], v2sqb, start=True, stop=True)
                nc.tensor.matmul(psQ[32:33, 0:256], onesc_sb, v2sq, start=True, stop=True)
                nc.tensor.matmul(psQ[:, 256:337], v1sqb[:, 0:H], w2bd, start=True, stop=True)
                nc.tensor.matmul(psQ[:, 337:418], v1sqb[:, H:S], w2bd, start=True, stop=True)
                nc.tensor.matmul(psQ[:, 418:419], v2sq[:, 0:H], onesc_sb, start=True, stop=True)
                nc.tensor.matmul(psQ[:, 419:420], v2sq[:, H:S], onesc_sb, start=True, stop=True)

                rs1a = mid.tile([H, 2 * 81], f32, name="rs1a")
                nc.vector.reciprocal(rs1a, psQ[:, 256:418])
                nc.scalar.sqrt(rs1a, rs1a)
                rs2y = mid.tile([H, 2], f32, name="rs2y")
                nc.vector.reciprocal(rs2y, psQ[:, 418:420])
                nc.scalar.sqrt(rs2y, rs2y)
                r2mf = mid.tile([L, 256], f32, name="r2mf")
                nc.vector.reciprocal(r2mf, psQ[0:L, 0:256])
                r2m = mid.tile([L, 256], bf16, name="r2m")
                nc.scalar.sqrt(r2m, r2mf)
                rrowf = mid.tile([1, 256], f32, name="rrowf")
                nc.vector.reciprocal(rrowf, psQ[32:33, 0:256])
                rrow = mid.tile([1, 256], f32, name="rrow")
                nc.scalar.sqrt(rrow, rrowf)

                # eps * ||v1|| (sign bias for m3)
                nv1e = mid.tile([H, 2], f32, name="nv1e")
                nc.scalar.sqrt(nv1e[:, 0:1], psQ[:, 336:337])
                nc.scalar.sqrt(nv1e[:, 1:2], psQ[:, 417:418])
                nc.scalar.mul(nv1e, nv1e, EPS)

                # ---- m1 ----
                tcolf = v2f[:, 255:256] if d == 0 else v2f[:, 0:1]
                sqt = mid.tile([H, 1], bf16, name="sqt")
                nc.scalar.square(sqt, tcolf)
                rhs1 = mid.tile([H, L], bf16, name="rhs1")
                nc.vector.tensor_scalar_mul(rhs1, w2bd[:, 0:20], tcolf)
                psSmall = psSmall_p.tile([H, 512], f32, name="psSmall")
                # psSmall: [c*100: num1 0:20 | num3 20:40 | num4 40:60 | n3 60:80 | n4 80:100]
                # [200:220 n2a-rep | 220:222 Gr | 222:242 (n2a row0) | 242:498 idxT row0]
                nc.tensor.matmul(psSmall[0:1, 222:242], sqt, w2bd[:, 0:20], start=True, stop=True)
                n2a_sb = mid.tile([1, L], bf16, name="n2a_sb")
                nc.scalar.copy(n2a_sb, psSmall[0:1, 222:242])
                nc.tensor.matmul(psSmall[:, 200:220], onesrb_sb, n2a_sb, start=True, stop=True)
                rs2rep = mid.tile([H, L], f32, name="rs2rep")
                nc.vector.reciprocal(rs2rep, psSmall[:, 200:220])
                nc.scalar.sqrt(rs2rep, rs2rep)
                nc.tensor.matmul(psSmall[:, 0:20], v1b[:, 0:H], rhs1, start=True, stop=True)
                nc.tensor.matmul(psSmall[:, 100:120], v1b[:, H:S], rhs1, start=True, stop=True)

                # ---- G (fp32, argmax) / GT (bf16, m3) ----
                # Pre-scale v2f columns by 1/||v2_j|| (replicated via stride-0 DMA) so
                # psG is the scaled-cos matrix directly -- no PSUM-x-PSUM TT needed.
                repG = mid.tile([H, 256], f32, name="repG")
                nc.sync.dma_start(out=repG, in_=rrow[0:1, :].unsqueeze(1).to_broadcast((1, H, 256)))
                v2fs = mid.tile([H, S], f32, name="v2fs")
                nc.gpsimd.tensor_tensor(v2fs, v2f, repG, op=OP.mult)
                psG = psG_p.tile([H, 512], f32, name="psG")
                nc.tensor.matmul(psG[:, 0:256], v1f[:, 0:H], v2fs, start=True, stop=True)
                nc.tensor.matmul(psG[:, 256:512], v1f[:, H:S], v2fs, start=True, stop=True)
                # GT lives in the psWork bank; GW/a4 overwrite it after GT_sb copies
                psWork = psWork_p.tile([H, 512], f32, name="psWork")
                nc.tensor.matmul(psWork[:, 0:256], v2b[:, 0:H], v1b, start=True, stop=True)
                nc.tensor.matmul(psWork[:, 256:512], v2b[:, H:S], v1b, start=True, stop=True)
                GT_sb = mid.tile([H, 2, S], bf16, name="GT_sb")
                nc.scalar.copy(GT_sb[:, 0, :], psWork[:, 0:256])
                nc.scalar.copy(GT_sb[:, 1, :], psWork[:, 256:512])

                # v2r rows (bf16) for GWT; y (fp32) for Gr sign
                v2rb = mid.tile([H, 2, H], bf16, name="v2rb")
                nc.vector.tensor_scalar_mul(v2rb[:, 0, :], v2nb[:, 0, :], rs2y[:, 0:1])
                nc.vector.tensor_scalar_mul(v2rb[:, 1, :], v2nb[:, 1, :], rs2y[:, 1:2])
                # GW at [0:256], att4T at [256:512]
                nc.tensor.matmul(psWork[:, 0:256], v2rb[:, 0, :], GT_sb[:, 0, :], start=True, stop=False)
                nc.tensor.matmul(psWork[:, 0:256], v2rb[:, 1, :], GT_sb[:, 1, :], start=False, stop=True)

                # y = sum_j v2_j / ||v2_j||  (all fp32: the m3 sign flips otherwise)
                psY = psSmall
                nc.tensor.matmul(psY[:, 220:221], v2n[:, 0, :], rs2y[:, 0:1], start=True, stop=False)
                nc.tensor.matmul(psY[:, 220:221], v2n[:, 1, :], rs2y[:, 1:2], start=False, stop=True)
                y_sb = mid.tile([H, 1], f32, name="y_sb")
                nc.scalar.copy(y_sb, psY[:, 220:221])
                nc.tensor.matmul(psY[:, 221:222], v1f[:, 0:H], y_sb, start=True, stop=True)
                nc.tensor.matmul(psY[:, 498:499], v1f[:, H:S], y_sb, start=True, stop=True)
                sgn0 = mid.tile([H, 1], f32, name="sgn0")
                nc.scalar.activation(sgn0, psY[:, 221:222], AF.Sign, bias=nv1e[:, 0:1], scale=1.0)
                sgn1 = mid.tile([H, 1], f32, name="sgn1")
                nc.scalar.activation(sgn1, psY[:, 498:499], AF.Sign, bias=nv1e[:, 1:2], scale=1.0)

                # ---- argmax head: top1 index of scaled G (all fp32) ----
                psM = psM_p.tile([H, 512], f32, name="psM")
                idxf = mid.tile([H, 2], f32, name="idxf")
                for c in range(2):
                    Gsc = mid.tile([H, 256], f32, name="Gsc")
                    nc.scalar.copy(Gsc, psG[:, 256 * c : 256 * c + 256])
                    top8 = mid.tile([H, 8], f32, name="top8")
                    nc.vector.max(top8, Gsc)
                    idx8 = mid.tile([H, 8], mybir.dt.uint32, name="idx8")
                    nc.vector.max_index(idx8, top8, Gsc)
                    nc.vector.tensor_copy(idxf[:, c : c + 1], idx8[:, 0:1])

                # m3 num/denominator matmuls (ready pre-loop).
                # ACT stages GW to SBUF bf16 once; Pool derives prod3 and sq3 from it.
                gwb = mid.tile([H, S], bf16, name="gwb")
                nc.scalar.copy(gwb, psWork[:, 0:256])
                prod3 = mid.tile([H, S], bf16, name="prod3")
                nc.gpsimd.tensor_tensor(prod3, v1b, gwb, op=OP.mult)
                sq3 = mid.tile([H, S], bf16, name="sq3")
                nc.gpsimd.tensor_tensor(sq3, gwb, gwb, op=OP.mult)
                for c in range(2):
                    base = 100 * c
                    sl = slice(c * H, c * H + H)
                    nc.tensor.matmul(psSmall[:, base + 20 : base + 40], prod3[:, sl], w2bd[:, 40:60], start=True, stop=True)
                    nc.tensor.matmul(psSmall[:, base + 60 : base + 80], sq3[:, sl], w2bd[:, 40:60], start=True, stop=True)

                # ---- m2 l-loop ----
                # Per 2 perspectives: M-pair (PE, K=20 one-hot matmul) -> psM bank,
                # DMA psM -> SBUF f32 (Pool may not touch PSUM), Pool v2s TT,
                # PE num matmuls, one strided DVE max drains each psNum to bf16.
                w2ed = w2e_sb[:, d, :]
                t2 = big.tile([H, L, 2], f32, name="t2")
                psNums = [
                    psNum_p.tile([H, 512], f32, name="psNumA"),
                    psNum_p.tile([H, 512], f32, name="psNumB"),
                    psNum_p.tile([H, 512], f32, name="psNumC"),
                ]
                NS = L // 2
                mb_t = [None] * NS
                for s in range(NS + 1):
                    if s < NS:
                        # M for l=2s (cols 0:256) and l=2s+1 (cols 256:512)
                        nc.tensor.matmul(psM[:, 0:256], w2ed[:, (2 * s) * H : (2 * s + 1) * H],
                                         r2m, start=True, stop=True)
                        nc.tensor.matmul(psM[:, 256:512], w2ed[:, (2 * s + 1) * H : (2 * s + 2) * H],
                                         r2m, start=True, stop=True)
                        mb = m2p.tile([H, 512], bf16, name="mb")
                        mb_t[s] = mb
                        nc.scalar.copy(mb, psM[:, 0:512])
                    if s >= 1:
                        t = s - 1
                        for j in range(2):
                            l = 2 * t + j
                            v2s = m2p.tile([H, S], bf16, name="v2s")
                            nc.gpsimd.tensor_tensor(v2s, mb_t[t][:, 256 * j : 256 * j + 256],
                                                    v2b, op=OP.mult)
                            psNum = psNums[l % 3]
                            nc.tensor.matmul(psNum[:, 0:256], v1b[:, 0:H], v2s, start=True, stop=True)
                            nc.tensor.matmul(psNum[:, 256:512], v1b[:, H:S], v2s, start=True, stop=True)
                            # drain psNum with one full max-reduce -> [H,2]
                            pv = psNum[:].rearrange("p (c j) -> p c j", c=2)
                            nc.vector.tensor_reduce(t2[:, l, :], pv, axis=AX.X, op=OP.max)

                # ---- post-loop: m4 index -> mask -> gather -> tails ----
                for c in range(2):
                    nc.tensor.transpose(psSmall[0:1, 242 + c * H : 242 + c * H + H],
                                        idxf[:, c : c + 1], ident_sb)
                idxT = mid.tile([1, 256], bf16, name="idxT")
                nc.scalar.copy(idxT, psSmall[0:1, 242:498])
                # replicate idx row via DMA; masks on Pool (all SBUF)
                idxrepS = mid.tile([H, 256], bf16, name="idxrepS")
                nc.sync.dma_start(out=idxrepS, in_=idxT[0:1, :].unsqueeze(1).to_broadcast((1, H, 256)))
                maskT0 = mid.tile([H, 256], bf16, name="maskT0")
                nc.vector.tensor_tensor(maskT0, idxrepS,
                                        iotab_sb[:, 0:1].to_broadcast((H, 256)), op=OP.is_equal)
                maskT1 = mid.tile([H, 256], bf16, name="maskT1")
                nc.vector.tensor_tensor(maskT1, idxrepS,
                                        iotab_sb[:, 1:2].to_broadcast((H, 256)), op=OP.is_equal)
                nc.tensor.matmul(psWork[:, 256:512], q0c[:, d, 0, :], maskT0, start=True, stop=False)
                nc.tensor.matmul(psWork[:, 256:512], q0c[:, d, 1, :], maskT1, start=False, stop=True)

                a4b = mid.tile([H, S], bf16, name="a4b")
                nc.scalar.copy(a4b, psWork[:, 256:512])
                prod4 = mid.tile([H, S], bf16, name="prod4")
                nc.gpsimd.tensor_tensor(prod4, v1b, a4b, op=OP.mult)
                sq4 = mid.tile([H, S], bf16, name="sq4")
                nc.gpsimd.tensor_tensor(sq4, a4b, a4b, op=OP.mult)
                for c in range(2):
                    base = 100 * c
                    sl = slice(c * H, c * H + H)
                    nc.tensor.matmul(psSmall[:, base + 40 : base + 60], prod4[:, sl], w2bd[:, 60:80], start=True, stop=True)
                    nc.tensor.matmul(psSmall[:, base + 80 : base + 100], sq4[:, sl], w2bd[:, 60:80], start=True, stop=True)

                # m2 finals (t2 is SBUF -> Pool)
                for c in range(2):
                    nc.gpsimd.tensor_tensor(outts[c][:, 40 + d * 20 : 40 + d * 20 + 20],
                                            t2[:, :, c], rs1a[:, 81 * c + 20 : 81 * c + 40], op=OP.mult)

                # m3 / m4 finals: one ACT copy of each psSmall chunk, Pool TTs after
                for c in range(2):
                    base = 100 * c
                    smf = mid.tile([H, 100], f32, name="smf")
                    nc.scalar.copy(smf, psSmall[:, base : base + 100])
                    rsq34 = mid.tile([H, 40], f32, name="rsq34")
                    nc.vector.reciprocal(rsq34, psSmall[:, base + 60 : base + 100])
                    nc.scalar.sqrt(rsq34, rsq34)
                    sgn = sgn0 if c == 0 else sgn1
                    t1 = mid.tile([H, L], f32, name="t1x")
                    nc.gpsimd.tensor_tensor(t1, smf[:, 0:20], rs1a[:, 81 * c : 81 * c + 20], op=OP.mult)
                    nc.gpsimd.tensor_tensor(outts[c][:, d * 20 : d * 20 + 20], t1, rs2rep, op=OP.mult)
                    t3 = mid.tile([H, L], f32, name="t3")
                    nc.gpsimd.tensor_tensor(t3, smf[:, 20:40],
                                            rs1a[:, 81 * c + 40 : 81 * c + 60], op=OP.mult)
                    t3b = mid.tile([H, L], f32, name="t3b")
                    nc.gpsimd.tensor_tensor(t3b, t3, rsq34[:, 0:20], op=OP.mult)
                    nc.scalar.mul(outts[c][:, 80 + d * 20 : 80 + d * 20 + 20], t3b, sgn)
                    t4 = mid.tile([H, L], f32, name="t4")
                    nc.gpsimd.tensor_tensor(t4, smf[:, 40:60],
                                            rs1a[:, 81 * c + 60 : 81 * c + 80], op=OP.mult)
                    nc.gpsimd.tensor_tensor(outts[c][:, 120 + d * 20 : 120 + d * 20 + 20],
                                            t4, rsq34[:, 20:40], op=OP.mult)

            # ---- store ----
            nc.sync.dma_start(out=out[b, 0:H, :], in_=outt0)
            nc.sync.dma_start(out=out[b, H:S, :], in_=outt1)

    return nc


def _prep_core_inputs(p, q, w_list, core):
    """Host-side layout prep for one core. Layout transforms + weight-only math."""
    import ml_dtypes

    bf16 = ml_dtypes.bfloat16
    sl = slice(core * BPC, (core + 1) * BPC)
    p8 = np.ascontiguousarray(p[sl])  # (BPC, 256, 256)
    q8 = np.ascontiguousarray(q[sl])
    pT = np.ascontiguousarray(p8.reshape(BPC, S, 2, H).transpose(0, 2, 3, 1))
    qT = np.ascontiguousarray(q8.reshape(BPC, S, 2, H).transpose(0, 2, 3, 1))
    qn = np.ascontiguousarray(q8.reshape(BPC, S, 2, H).transpose(0, 2, 1, 3))
    q0n = np.ascontiguousarray(q[0].reshape(S, 2, H).transpose(1, 0, 2))  # (2, S, H)

    w2T = np.empty((2, H, 81), np.float32)
    w2E = np.zeros((2, L, L * H), np.float32)
    for d in range(2):
        ws = w_list[d::2]  # fw: w1,w3,w5,w7 ; bw: w2,w4,w6,w8
        cat = np.concatenate([w * w for w in ws] + [np.ones((1, H), np.float32)], 0)
        w2T[d] = cat.T
        w2m2 = ws[1] * ws[1]  # (L, H) m2 weights squared
        for l in range(L):
            w2E[d, l, l * H : (l + 1) * H] = w2m2[l]
    iota2 = np.stack([np.arange(H, dtype=np.float32), np.arange(H, 2 * H, dtype=np.float32)], 1)

    return {
        "pTb": pT.astype(bf16),
        "qTb": qT.astype(bf16),
        "pTf": pT,
        "qTf": qT,
        "qn": qn,
        "qnb": qn.astype(bf16),
        "q0nb": q0n.astype(bf16),
        "w2T": w2T,
        "w2Tb": w2T.astype(bf16),
        "w2E": w2E.astype(bf16),
        "iota2": np.ascontiguousarray(iota2),
        "onesr": np.ones((1, H), np.float32),
        "onesrb": np.ones((1, H), bf16),
        "onesc": np.ones((H, 1), np.float32),
        "ident": np.eye(H, dtype=np.float32),
    }


def _legalize_bir(bir_bytes):
    """Move all but one sync-wait of each instruction onto inserted same-engine Drains
    (this neuronxcc build rejects >1 sync wait per instruction)."""
    import json as _json

    d = _json.loads(bir_bytes)
    n = 0
    for fnd in d["functions"]:
        for blk in fnd["blocks"]:
            insts = blk.get("instructions") or []
            out = []
            for ins in insts:
                si = ins.get("sync_info") or {}
                w = si.get("on_wait") or []
                if len(w) > 1:
                    for extra in w[:-1]:
                        out.append(
                            {
                                "debug": ins.get("debug", 0),
                                "engine": ins.get("engine"),
                                "ins": [],
                                "outs": [],
                                "is_reset_sema": False,
                                "name": f"I-legalw-{n}",
                                "opcode": "Drain",
                                "sync_info": {"on_update": [], "on_wait": [extra]},
                            }
                        )
                        n += 1
                    si["on_wait"] = [w[-1]]
                out.append(ins)
            blk["instructions"] = out
    return _json.dumps(d).encode(), n


def _install_legalizer():
    if _cache.get("legalizer"):
        return
    from concourse import bass2jax, bass_utils

    orig = bass_utils.compile_bir_kernel

    def patched(bir_json, tmpdir, neff_name="file.neff"):
        fixed, n = _legalize_bir(bir_json)
        return orig(fixed, tmpdir, neff_name)

    bass2jax.compile_bir_kernel = patched
    _cache["legalizer"] = True


def _get_runner():
    if "runner" in _cache:
        return _cache["runner"]

    import jax
    from jax.sharding import Mesh, PartitionSpec
    from jax.experimental.shard_map import shard_map

    import concourse.mybir as mybir
    from concourse import bass2jax

    if "nc" not in _cache:
        _cache["nc"] = _build_bass()
    nc = _cache["nc"]

    bass2jax.install_neuronx_cc_hook()
    _install_legalizer()
    assert nc.dbg_addr is None
    partition_name = nc.partition_id_tensor.name if nc.partition_id_tensor else None

    in_names, out_names, out_avals, zero_outs = [], [], [], []
    for alloc in nc.m.functions[0].allocations:
        if not isinstance(alloc, mybir.MemoryLocationSet):
            continue
        name = alloc.memorylocations[0].name
        if alloc.kind == "ExternalInput":
            if name != partition_name:
                in_names.append(name)
        elif alloc.kind == "ExternalOutput":
            out_names.append(name)
            shape = tuple(alloc.tensor_shape)
            dtype = mybir.dt.np(alloc.dtype)
            out_avals.append(jax.core.ShapedArray(shape, dtype))
            zero_outs.append(np.zeros(shape, dtype))
    n_params = len(in_names)
    all_names = in_names + out_names
    if partition_name is not None:
        all_names = all_names + [partition_name]

    def _body(*args):
        operands = list(args)
        if partition_name is not None:
            operands.append(bass2jax.partition_id_tensor())
        outs = bass2jax._bass_exec_p.bind(
            *operands,
            out_avals=tuple(out_avals),
            in_names=tuple(all_names),
            out_names=tuple(out_names),
            lowering_input_output_aliases=(),
            sim_require_finite=True,
            sim_require_nnan=True,
            nc=nc,
        )
        return tuple(outs)

    devices = jax.devices()[:NCORES]
    mesh = Mesh(np.asarray(devices), ("core",))
    sharded = jax.jit(
        shard_map(
            _body,
            mesh=mesh,
            in_specs=(PartitionSpec("core"),) * (n_params + len(out_avals)),
            out_specs=(PartitionSpec("core"),) * len(out_avals),
            check_rep=False,
        ),
        donate_argnums=tuple(range(n_params, n_params + len(out_avals))),
        keep_unused=True,
    )
    runner = {
        "jax": jax,
        "sharded": sharded,
        "in_names": in_names,
        "out_names": out_names,
        "out_avals": out_avals,
        "zero_outs": zero_outs,
        "n_params": n_params,
    }
    _cache["runner"] = runner
    return runner


def kernel(p, q, w1, w2, w3, w4, w5, w6, w7, w8, _time_iters=0):
    p = np.asarray(p, dtype=np.float32)
    q = np.asarray(q, dtype=np.float32)
    w_list = [np.asarray(w, dtype=np.float32) for w in (w1, w2, w3, w4, w5, w6, w7, w8)]

    r = _get_runner()
    jax = r["jax"]
    in_maps = [_prep_core_inputs(p, q, w_list, c) for c in range(NCORES)]
    concat_in = [
        np.concatenate([in_maps[c][name] for c in range(NCORES)], 0)
        for name in r["in_names"]
    ]
    concat_zeros = [
        np.zeros((NCORES * z.shape[0], *z.shape[1:]), z.dtype) for z in r["zero_outs"]
    ]
    out_arrs = r["sharded"](*concat_in, *concat_zeros)
    jax.block_until_ready(out_arrs)
    out = np.asarray(out_arrs[r["out_names"].index("out")])  # (64, 256, 160)

    if _time_iters:
        import time

        dev_in = [jax.device_put(a) for a in concat_in]
        jax.block_until_ready(dev_in)
        times = []
        for _ in range(_time_iters):
            zeros = [
                jax.device_put(np.zeros((NCORES * z.shape[0], *z.shape[1:]), z.dtype))
                for z in r["zero_outs"]
            ]
            jax.block_until_ready(zeros)
            t0 = time.perf_counter()
            o = r["sharded"](*dev_in, *zeros)
            jax.block_until_ready(o)
            times.append(time.perf_counter() - t0)
        kernel.last_exec_time_ns = int(min(times) * 1e9)
        kernel.all_times_ns = [int(t * 1e9) for t in times]
    return out


# revision 8
# speedup vs baseline: 1.4194x; 1.4194x over previous
"""BiMPM matching-layer kernel for Trainium2 (8 NeuronCores, pure data parallel).

v2: restructured for engine balance.
 - All matmuls bf16 except the m4-argmax chain (G, rrow, rrep) which must be
   fp32-exact so the argmax matches the reference (a flipped index gathers a
   completely different q0 row).
 - m2 (max-pooling): per perspective l, scale v2 by w2_l (rows) and 1/n2_l
   (cols) BEFORE the num matmul.  The [H,256] column-scale tile M_l =
   w2_l(h) * r2(l,j) is produced by one K=20 matmul against a host constant
   W2E whose l-th block is the one-hot row l scaled by w2_l.  The max over q
   runs hierarchically: Pool tensor_max halves PSUM (L1), DVE batched
   tensor_max (L2), DVE batched tensor_reduce tail.
Output (B, S1, 8*L), L=20. Batch 64 -> 8 per core.
"""
import numpy as np

B, S, H, L = 64, 256, 128, 20
NCORES = 8
BPC = B // NCORES
EPS = 1e-8

_cache = {}


def _build_bass():
    from contextlib import ExitStack

    import concourse.bass as bass
    import concourse.tile as tile
    from concourse import mybir

    f32 = mybir.dt.float32
    bf16 = mybir.dt.bfloat16
    AF = mybir.ActivationFunctionType
    OP = mybir.AluOpType
    AX = mybir.AxisListType

    nc = bass.Bass()

    # DRAM I/O (per core)
    pTb = nc.dram_tensor("pTb", [BPC, 2, H, S], bf16, kind="ExternalInput")
    qTb = nc.dram_tensor("qTb", [BPC, 2, H, S], bf16, kind="ExternalInput")
    pTf = nc.dram_tensor("pTf", [BPC, 2, H, S], f32, kind="ExternalInput")
    qTf = nc.dram_tensor("qTf", [BPC, 2, H, S], f32, kind="ExternalInput")
    qn = nc.dram_tensor("qn", [BPC, 2, S, H], f32, kind="ExternalInput")
    qnb = nc.dram_tensor("qnb", [BPC, 2, S, H], bf16, kind="ExternalInput")
    q0nb = nc.dram_tensor("q0nb", [2, S, H], bf16, kind="ExternalInput")
    w2T = nc.dram_tensor("w2T", [2, H, 81], f32, kind="ExternalInput")
    w2Tb = nc.dram_tensor("w2Tb", [2, H, 81], bf16, kind="ExternalInput")
    w2E = nc.dram_tensor("w2E", [2, L, L * H], bf16, kind="ExternalInput")
    iota2 = nc.dram_tensor("iota2", [H, 2], f32, kind="ExternalInput")
    onesr = nc.dram_tensor("onesr", [1, H], f32, kind="ExternalInput")
    onesrb = nc.dram_tensor("onesrb", [1, H], bf16, kind="ExternalInput")
    onesc = nc.dram_tensor("onesc", [H, 1], f32, kind="ExternalInput")
    ident = nc.dram_tensor("ident", [H, H], f32, kind="ExternalInput")
    out = nc.dram_tensor("out", [BPC, S, 8 * L], f32, kind="ExternalOutput")

    with tile.TileContext(nc) as tc, ExitStack() as ctx:
        cpool = ctx.enter_context(tc.tile_pool(name="consts", bufs=1))
        inp = ctx.enter_context(tc.tile_pool(name="inp", bufs=3))
        mid = ctx.enter_context(tc.tile_pool(name="mid", bufs=2))
        m2p = ctx.enter_context(tc.tile_pool(name="m2p", bufs=3))
        big = ctx.enter_context(tc.tile_pool(name="big", bufs=2))
        outp = ctx.enter_context(tc.tile_pool(name="outp", bufs=3))
        # PSUM (8 banks): each [H,512] f32 tile = 1 bank
        psQ_p = ctx.enter_context(tc.tile_pool(name="psQ", bufs=1, space="PSUM"))
        psG_p = ctx.enter_context(tc.tile_pool(name="psG", bufs=1, space="PSUM"))
        psWork_p = ctx.enter_context(tc.tile_pool(name="psWork", bufs=1, space="PSUM"))
        psSmall_p = ctx.enter_context(tc.tile_pool(name="psSmall", bufs=1, space="PSUM"))
        psM_p = ctx.enter_context(tc.tile_pool(name="psM", bufs=1, space="PSUM"))
        psNum_p = ctx.enter_context(tc.tile_pool(name="psNum", bufs=1, space="PSUM"))

        # ---- constants ----
        w2_sb = cpool.tile([H, 2, 81], f32)
        nc.sync.dma_start(out=w2_sb, in_=w2T[:].rearrange("d h c -> h d c"))
        w2b_sb = cpool.tile([H, 2, 81], bf16)
        nc.sync.dma_start(out=w2b_sb, in_=w2Tb[:].rearrange("d h c -> h d c"))
        w2e_sb = cpool.tile([L, 2, L * H], bf16)
        nc.sync.dma_start(out=w2e_sb, in_=w2E[:].rearrange("d l c -> l d c"))
        q0c = cpool.tile([H, 2, 2, H], bf16)
        nc.sync.dma_start(out=q0c, in_=q0nb[:].rearrange("d (c q) h -> q d c h", c=2))
        iota_sb = cpool.tile([H, 2], f32)
        nc.sync.dma_start(out=iota_sb, in_=iota2[:])
        iotab_sb = cpool.tile([H, 2], bf16)
        nc.vector.tensor_copy(iotab_sb, iota_sb)
        onesr_sb = cpool.tile([1, H], f32)
        nc.sync.dma_start(out=onesr_sb, in_=onesr[:])
        onesrb_sb = cpool.tile([1, H], bf16)
        nc.sync.dma_start(out=onesrb_sb, in_=onesrb[:])
        onesc_sb = cpool.tile([H, 1], f32)
        nc.sync.dma_start(out=onesc_sb, in_=onesc[:])
        ident_sb = cpool.tile([H, H], f32)
        nc.sync.dma_start(out=ident_sb, in_=ident[:])

        for b in range(BPC):
            outt0 = outp.tile([H, 8 * L], f32, name="outt0")
            outt1 = outp.tile([H, 8 * L], f32, name="outt1")
            outts = (outt0, outt1)
            for d in range(2):
                w2d = w2_sb[:, d, :]
                w2bd = w2b_sb[:, d, :]
                # ---- loads ----
                v1b = inp.tile([H, S], bf16, name="v1b")
                nc.sync.dma_start(out=v1b, in_=pTb[b, d])
                v2b = inp.tile([H, S], bf16, name="v2b")
                nc.sync.dma_start(out=v2b, in_=qTb[b, d])
                v1f = inp.tile([H, S], f32, name="v1f")
                nc.sync.dma_start(out=v1f, in_=pTf[b, d])
                v2f = inp.tile([H, S], f32, name="v2f")
                nc.sync.dma_start(out=v2f, in_=qTf[b, d])
                v2n = inp.tile([H, 2, H], f32, name="v2n")
                nc.sync.dma_start(out=v2n, in_=qn[b, d].rearrange("(c q) h -> q c h", c=2))
                v2nb = inp.tile([H, 2, H], bf16, name="v2nb")
                nc.sync.dma_start(out=v2nb, in_=qnb[b, d].rearrange("(c q) h -> q c h", c=2))

                # ---- squares (Pool: SBUF-only ops, keeps ACT free) ----
                v1sqb = mid.tile([H, S], bf16, name="v1sqb")
                nc.gpsimd.tensor_tensor(v1sqb, v1b, v1b, op=OP.mult)
                v2sqb = mid.tile([H, S], bf16, name="v2sqb")
                nc.gpsimd.tensor_tensor(v2sqb, v2b, v2b, op=OP.mult)
                v2sq = mid.tile([H, S], f32, name="v2sq")
                nc.scalar.square(v2sq, v2f)

                # ---- norms (one bank) ----
                # psQ: [rows 0:20 cols 0:256 n2m2T | row 32 cols 0:256 rrow |
                #       cols 256:418 n1 (2x81) | cols 418:420 ||v2_j||^2 fp32]
                psQ = psQ_p.tile([H, 512], f32, name="psQ")
                nc.tensor.matmul(psQ[0:L, 0:256], w2bd[:, 20:40], v2sqb, start=True, stop=True)
                nc.tensor.matmul(psQ[32:33, 0:256], onesc_sb, v2sq, start=True, stop=True)
                nc.tensor.matmul(psQ[:, 256:337], v1sqb[:, 0:H], w2bd, start=True, stop=True)
                nc.tensor.matmul(psQ[:, 337:418], v1sqb[:, H:S], w2bd, start=True, stop=True)
                nc.tensor.matmul(psQ[:, 418:419], v2sq[:, 0:H], onesc_sb, start=True, stop=True)
                nc.tensor.matmul(psQ[:, 419:420], v2sq[:, H:S], onesc_sb, start=True, stop=True)

                rs1a = mid.tile([H, 2 * 81], f32, name="rs1a")
                nc.vector.reciprocal(rs1a, psQ[:, 256:418])
                nc.scalar.sqrt(rs1a, rs1a)
                rs2y = mid.tile([H, 2], f32, name="rs2y")
                nc.vector.reciprocal(rs2y, psQ[:, 418:420])
                nc.scalar.sqrt(rs2y, rs2y)
                r2mf = mid.tile([L, 256], f32, name="r2mf")
                nc.vector.reciprocal(r2mf, psQ[0:L, 0:256])
                r2m = mid.tile([L, 256], bf16, name="r2m")
                nc.scalar.sqrt(r2m, r2mf)
                rrowf = mid.tile([1, 256], f32, name="rrowf")
                nc.vector.reciprocal(rrowf, psQ[32:33, 0:256])
                rrow = mid.tile([1, 256], f32, name="rrow")
                nc.scalar.sqrt(rrow, rrowf)

                # eps * ||v1|| (sign bias for m3)
                nv1e = mid.tile([H, 2], f32, name="nv1e")
                nc.scalar.sqrt(nv1e[:, 0:1], psQ[:, 336:337])
                nc.scalar.sqrt(nv1e[:, 1:2], psQ[:, 417:418])
                nc.scalar.mul(nv1e, nv1e, EPS)

                # ---- m1 ----
                tcolf = v2f[:, 255:256] if d == 0 else v2f[:, 0:1]
                sqt = mid.tile([H, 1], bf16, name="sqt")
                nc.scalar.square(sqt, tcolf)
                rhs1 = mid.tile([H, L], bf16, name="rhs1")
                nc.vector.tensor_scalar_mul(rhs1, w2bd[:, 0:20], tcolf)
                psSmall = psSmall_p.tile([H, 512], f32, name="psSmall")
                # psSmall: [c*100: num1 0:20 | num3 20:40 | num4 40:60 | n3 60:80 | n4 80:100]
                # [200:220 n2a-rep | 220:222 Gr | 222:242 (n2a row0) | 242:498 idxT row0]
                nc.tensor.matmul(psSmall[0:1, 222:242], sqt, w2bd[:, 0:20], start=True, stop=True)
                n2a_sb = mid.tile([1, L], bf16, name="n2a_sb")
                nc.scalar.copy(n2a_sb, psSmall[0:1, 222:242])
                nc.tensor.matmul(psSmall[:, 200:220], onesrb_sb, n2a_sb, start=True, stop=True)
                rs2rep = mid.tile([H, L], f32, name="rs2rep")
                nc.vector.reciprocal(rs2rep, psSmall[:, 200:220])
                nc.scalar.sqrt(rs2rep, rs2rep)
                nc.tensor.matmul(psSmall[:, 0:20], v1b[:, 0:H], rhs1, start=True, stop=True)
                nc.tensor.matmul(psSmall[:, 100:120], v1b[:, H:S], rhs1, start=True, stop=True)

                # ---- G (fp32, argmax) / GT (bf16, m3) ----
                # Pre-scale v2f columns by 1/||v2_j|| (replicated via stride-0 DMA) so
                # psG is the scaled-cos matrix directly -- no PSUM-x-PSUM TT needed.
                repG = mid.tile([H, 256], f32, name="repG")
                nc.sync.dma_start(out=repG, in_=rrow[0:1, :].unsqueeze(1).to_broadcast((1, H, 256)))
                v2fs = mid.tile([H, S], f32, name="v2fs")
                nc.gpsimd.tensor_tensor(v2fs, v2f, repG, op=OP.mult)
                psG = psG_p.tile([H, 512], f32, name="psG")
                nc.tensor.matmul(psG[:, 0:256], v1f[:, 0:H], v2fs, start=True, stop=True)
                nc.tensor.matmul(psG[:, 256:512], v1f[:, H:S], v2fs, start=True, stop=True)
                # GT lives in the psWork bank; GW/a4 overwrite it after GT_sb copies
                psWork = psWork_p.tile([H, 512], f32, name="psWork")
                nc.tensor.matmul(psWork[:, 0:256], v2b[:, 0:H], v1b, start=True, stop=True)
                nc.tensor.matmul(psWork[:, 256:512], v2b[:, H:S], v1b, start=True, stop=True)
                GT_sb = mid.tile([H, 2, S], bf16, name="GT_sb")
                nc.scalar.copy(GT_sb[:, 0, :], psWork[:, 0:256])
                nc.scalar.copy(GT_sb[:, 1, :], psWork[:, 256:512])

                # v2r rows (bf16) for GWT; y (fp32) for Gr sign
                v2rb = mid.tile([H, 2, H], bf16, name="v2rb")
                nc.vector.tensor_scalar_mul(v2rb[:, 0, :], v2nb[:, 0, :], rs2y[:, 0:1])
                nc.vector.tensor_scalar_mul(v2rb[:, 1, :], v2nb[:, 1, :], rs2y[:, 1:2])
                # GW at [0:256], att4T at [256:512]
                nc.tensor.matmul(psWork[:, 0:256], v2rb[:, 0, :], GT_sb[:, 0, :], start=True, stop=False)
                nc.tensor.matmul(psWork[:, 0:256], v2rb[:, 1, :], GT_sb[:, 1, :], start=False, stop=True)

                # y = sum_j v2_j / ||v2_j||  (all fp32: the m3 sign flips otherwise)
                psY = psSmall
                nc.tensor.matmul(psY[:, 220:221], v2n[:, 0, :], rs2y[:, 0:1], start=True, stop=False)
                nc.tensor.matmul(psY[:, 220:221], v2n[:, 1, :], rs2y[:, 1:2], start=False, stop=True)
                y_sb = mid.tile([H, 1], f32, name="y_sb")
                nc.scalar.copy(y_sb, psY[:, 220:221])
                nc.tensor.matmul(psY[:, 221:222], v1f[:, 0:H], y_sb, start=True, stop=True)
                nc.tensor.matmul(psY[:, 498:499], v1f[:, H:S], y_sb, start=True, stop=True)
                sgn0 = mid.tile([H, 1], f32, name="sgn0")
                nc.scalar.activation(sgn0, psY[:, 221:222], AF.Sign, bias=nv1e[:, 0:1], scale=1.0)
                sgn1 = mid.tile([H, 1], f32, name="sgn1")
                nc.scalar.activation(sgn1, psY[:, 498:499], AF.Sign, bias=nv1e[:, 1:2], scale=1.0)

                # ---- argmax head: top1 index of scaled G (all fp32) ----
                psM = psM_p.tile([H, 512], f32, name="psM")
                idxf = mid.tile([H, 2], f32, name="idxf")
                for c in range(2):
                    Gsc = mid.tile([H, 256], f32, name="Gsc")
                    nc.scalar.copy(Gsc, psG[:, 256 * c : 256 * c + 256])
                    top8 = mid.tile([H, 8], f32, name="top8")
                    nc.vector.max(top8, Gsc)
                    idx8 = mid.tile([H, 8], mybir.dt.uint32, name="idx8")
                    nc.vector.max_index(idx8, top8, Gsc)
                    nc.vector.tensor_copy(idxf[:, c : c + 1], idx8[:, 0:1])

                # m3 num/denominator matmuls (ready pre-loop).
                # ACT stages GW to SBUF bf16 once; Pool derives prod3 and sq3 from it.
                gwb = mid.tile([H, S], bf16, name="gwb")
                nc.scalar.copy(gwb, psWork[:, 0:256])
                prod3 = mid.tile([H, S], bf16, name="prod3")
                nc.gpsimd.tensor_tensor(prod3, v1b, gwb, op=OP.mult)
                sq3 = mid.tile([H, S], bf16, name="sq3")
                nc.gpsimd.tensor_tensor(sq3, gwb, gwb, op=OP.mult)
                for c in range(2):
                    base = 100 * c
                    sl = slice(c * H, c * H + H)
                    nc.tensor.matmul(psSmall[:, base + 20 : base + 40], prod3[:, sl], w2bd[:, 40:60], start=True, stop=True)
                    nc.tensor.matmul(psSmall[:, base + 60 : base + 80], sq3[:, sl], w2bd[:, 40:60], start=True, stop=True)

                # ---- m2 l-loop ----
                # Per 2 perspectives: M-pair (PE, K=20 one-hot matmul) -> psM bank,
                # DMA psM -> SBUF f32 (Pool may not touch PSUM), Pool v2s TT,
                # PE num matmuls, one strided DVE max drains each psNum to bf16.
                w2ed = w2e_sb[:, d, :]
                t2 = big.tile([H, L, 2], f32, name="t2")
                psNums = [
                    psNum_p.tile([H, 512], f32, name="psNumA"),
                    psNum_p.tile([H, 512], f32, name="psNumB"),
                    psNum_p.tile([H, 512], f32, name="psNumC"),
                ]
                NS = L // 2
                mb_t = [None] * NS
                for s in range(NS + 1):
                    if s < NS:
                        # M for l=2s (cols 0:256) and l=2s+1 (cols 256:512)
                        nc.tensor.matmul(psM[:, 0:256], w2ed[:, (2 * s) * H : (2 * s + 1) * H],
                                         r2m, start=True, stop=True)
                        nc.tensor.matmul(psM[:, 256:512], w2ed[:, (2 * s + 1) * H : (2 * s + 2) * H],
                                         r2m, start=True, stop=True)
                        mb = m2p.tile([H, 512], bf16, name="mb")
                        mb_t[s] = mb
                        nc.scalar.copy(mb, psM[:, 0:512])
                    if s >= 1:
                        t = s - 1
                        for j in range(2):
                            l = 2 * t + j
                            v2s = m2p.tile([H, S], bf16, name="v2s")
                            nc.gpsimd.tensor_tensor(v2s, mb_t[t][:, 256 * j : 256 * j + 256],
                                                    v2b, op=OP.mult)
                            psNum = psNums[l % 3]
                            nc.tensor.matmul(psNum[:, 0:256], v1b[:, 0:H], v2s, start=True, stop=True)
                            nc.tensor.matmul(psNum[:, 256:512], v1b[:, H:S], v2s, start=True, stop=True)
                            if l in (2, 5, 8, 12, 15, 18):
                                # drain via ACT copy; max tree on DVE at 2x from bf16 SBUF
                                stg = m2p.tile([H, 2, 2, H], bf16, name="stg")
                                nc.scalar.copy(stg, psNum[:].rearrange(
                                    "p (c two j) -> p c two j", c=2, two=2))
                                tm = m2p.tile([H, 2, H], bf16, name="tm")
                                nc.vector.tensor_max(tm, stg[:, :, 0, :], stg[:, :, 1, :])
                                nc.vector.tensor_reduce(t2[:, l, :], tm, axis=AX.X, op=OP.max)
                            else:
                                # drain psNum with one full max-reduce -> [H,2]
                                pv = psNum[:].rearrange("p (c j) -> p c j", c=2)
                                nc.vector.tensor_reduce(t2[:, l, :], pv, axis=AX.X, op=OP.max)

                # ---- post-loop: m4 index -> mask -> gather -> tails ----
                for c in range(2):
                    nc.tensor.transpose(psSmall[0:1, 242 + c * H : 242 + c * H + H],
                                        idxf[:, c : c + 1], ident_sb)
                idxT = mid.tile([1, 256], bf16, name="idxT")
                nc.scalar.copy(idxT, psSmall[0:1, 242:498])
                # replicate idx row via DMA; masks on Pool (all SBUF)
                idxrepS = mid.tile([H, 256], bf16, name="idxrepS")
                nc.sync.dma_start(out=idxrepS, in_=idxT[0:1, :].unsqueeze(1).to_broadcast((1, H, 256)))
                maskT0 = mid.tile([H, 256], bf16, name="maskT0")
                nc.vector.tensor_tensor(maskT0, idxrepS,
                                        iotab_sb[:, 0:1].to_broadcast((H, 256)), op=OP.is_equal)
                maskT1 = mid.tile([H, 256], bf16, name="maskT1")
                nc.vector.tensor_tensor(maskT1, idxrepS,
                                        iotab_sb[:, 1:2].to_broadcast((H, 256)), op=OP.is_equal)
                nc.tensor.matmul(psWork[:, 256:512], q0c[:, d, 0, :], maskT0, start=True, stop=False)
                nc.tensor.matmul(psWork[:, 256:512], q0c[:, d, 1, :], maskT1, start=False, stop=True)

                a4b = mid.tile([H, S], bf16, name="a4b")
                nc.scalar.copy(a4b, psWork[:, 256:512])
                prod4 = mid.tile([H, S], bf16, name="prod4")
                nc.gpsimd.tensor_tensor(prod4, v1b, a4b, op=OP.mult)
                sq4 = mid.tile([H, S], bf16, name="sq4")
                nc.gpsimd.tensor_tensor(sq4, a4b, a4b, op=OP.mult)
                for c in range(2):
                    base = 100 * c
                    sl = slice(c * H, c * H + H)
                    nc.tensor.matmul(psSmall[:, base + 40 : base + 60], prod4[:, sl], w2bd[:, 60:80], start=True, stop=True)
                    nc.tensor.matmul(psSmall[:, base + 80 : base + 100], sq4[:, sl], w2bd[:, 60:80], start=True, stop=True)

                # m2 finals (t2 is SBUF -> Pool)
                for c in range(2):
                    nc.gpsimd.tensor_tensor(outts[c][:, 40 + d * 20 : 40 + d * 20 + 20],
                                            t2[:, :, c], rs1a[:, 81 * c + 20 : 81 * c + 40], op=OP.mult)

                # m3 / m4 finals: one ACT copy of each psSmall chunk, Pool TTs after
                for c in range(2):
                    base = 100 * c
                    smf = mid.tile([H, 100], f32, name="smf")
                    nc.scalar.copy(smf, psSmall[:, base : base + 100])
                    rsq34 = mid.tile([H, 40], f32, name="rsq34")
                    nc.vector.reciprocal(rsq34, psSmall[:, base + 60 : base + 100])
                    nc.scalar.sqrt(rsq34, rsq34)
                    sgn = sgn0 if c == 0 else sgn1
                    t1 = mid.tile([H, L], f32, name="t1x")
                    nc.gpsimd.tensor_tensor(t1, smf[:, 0:20], rs1a[:, 81 * c : 81 * c + 20], op=OP.mult)
                    nc.gpsimd.tensor_tensor(outts[c][:, d * 20 : d * 20 + 20], t1, rs2rep, op=OP.mult)
                    t3 = mid.tile([H, L], f32, name="t3")
                    nc.gpsimd.tensor_tensor(t3, smf[:, 20:40],
                                            rs1a[:, 81 * c + 40 : 81 * c + 60], op=OP.mult)
                    t3b = mid.tile([H, L], f32, name="t3b")
                    nc.gpsimd.tensor_tensor(t3b, t3, rsq34[:, 0:20], op=OP.mult)
                    nc.scalar.mul(outts[c][:, 80 + d * 20 : 80 + d * 20 + 20], t3b, sgn)
                    t4 = mid.tile([H, L], f32, name="t4")
                    nc.gpsimd.tensor_tensor(t4, smf[:, 40:60],
                                            rs1a[:, 81 * c + 60 : 81 * c + 80], op=OP.mult)
                    nc.gpsimd.tensor_tensor(outts[c][:, 120 + d * 20 : 120 + d * 20 + 20],
                                            t4, rsq34[:, 20:40], op=OP.mult)

            # ---- store ----
            nc.sync.dma_start(out=out[b, 0:H, :], in_=outt0)
            nc.sync.dma_start(out=out[b, H:S, :], in_=outt1)

    return nc


def _prep_core_inputs(p, q, w_list, core):
    """Host-side layout prep for one core. Layout transforms + weight-only math."""
    import ml_dtypes

    bf16 = ml_dtypes.bfloat16
    sl = slice(core * BPC, (core + 1) * BPC)
    p8 = np.ascontiguousarray(p[sl])  # (BPC, 256, 256)
    q8 = np.ascontiguousarray(q[sl])
    pT = np.ascontiguousarray(p8.reshape(BPC, S, 2, H).transpose(0, 2, 3, 1))
    qT = np.ascontiguousarray(q8.reshape(BPC, S, 2, H).transpose(0, 2, 3, 1))
    qn = np.ascontiguousarray(q8.reshape(BPC, S, 2, H).transpose(0, 2, 1, 3))
    q0n = np.ascontiguousarray(q[0].reshape(S, 2, H).transpose(1, 0, 2))  # (2, S, H)

    w2T = np.empty((2, H, 81), np.float32)
    w2E = np.zeros((2, L, L * H), np.float32)
    for d in range(2):
        ws = w_list[d::2]  # fw: w1,w3,w5,w7 ; bw: w2,w4,w6,w8
        cat = np.concatenate([w * w for w in ws] + [np.ones((1, H), np.float32)], 0)
        w2T[d] = cat.T
        w2m2 = ws[1] * ws[1]  # (L, H) m2 weights squared
        for l in range(L):
            w2E[d, l, l * H : (l + 1) * H] = w2m2[l]
    iota2 = np.stack([np.arange(H, dtype=np.float32), np.arange(H, 2 * H, dtype=np.float32)], 1)

    return {
        "pTb": pT.astype(bf16),
        "qTb": qT.astype(bf16),
        "pTf": pT,
        "qTf": qT,
        "qn": qn,
        "qnb": qn.astype(bf16),
        "q0nb": q0n.astype(bf16),
        "w2T": w2T,
        "w2Tb": w2T.astype(bf16),
        "w2E": w2E.astype(bf16),
        "iota2": np.ascontiguousarray(iota2),
        "onesr": np.ones((1, H), np.float32),
        "onesrb": np.ones((1, H), bf16),
        "onesc": np.ones((H, 1), np.float32),
        "ident": np.eye(H, dtype=np.float32),
    }


def _legalize_bir(bir_bytes):
    """Move all but one sync-wait of each instruction onto inserted same-engine Drains
    (this neuronxcc build rejects >1 sync wait per instruction)."""
    import json as _json

    d = _json.loads(bir_bytes)
    n = 0
    for fnd in d["functions"]:
        for blk in fnd["blocks"]:
            insts = blk.get("instructions") or []
            out = []
            for ins in insts:
                si = ins.get("sync_info") or {}
                w = si.get("on_wait") or []
                if len(w) > 1:
                    for extra in w[:-1]:
                        out.append(
                            {
                                "debug": ins.get("debug", 0),
                                "engine": ins.get("engine"),
                                "ins": [],
                                "outs": [],
                                "is_reset_sema": False,
                                "name": f"I-legalw-{n}",
                                "opcode": "Drain",
                                "sync_info": {"on_update": [], "on_wait": [extra]},
                            }
                        )
                        n += 1
                    si["on_wait"] = [w[-1]]
                out.append(ins)
            blk["instructions"] = out
    return _json.dumps(d).encode(), n


def _install_legalizer():
    if _cache.get("legalizer"):
        return
    from concourse import bass2jax, bass_utils

    orig = bass_utils.compile_bir_kernel

    def patched(bir_json, tmpdir, neff_name="file.neff"):
        fixed, n = _legalize_bir(bir_json)
        return orig(fixed, tmpdir, neff_name)

    bass2jax.compile_bir_kernel = patched
    _cache["legalizer"] = True


def _get_runner():
    if "runner" in _cache:
        return _cache["runner"]

    import jax
    from jax.sharding import Mesh, PartitionSpec
    from jax.experimental.shard_map import shard_map

    import concourse.mybir as mybir
    from concourse import bass2jax

    if "nc" not in _cache:
        _cache["nc"] = _build_bass()
    nc = _cache["nc"]

    bass2jax.install_neuronx_cc_hook()
    _install_legalizer()
    assert nc.dbg_addr is None
    partition_name = nc.partition_id_tensor.name if nc.partition_id_tensor else None

    in_names, out_names, out_avals, zero_outs = [], [], [], []
    for alloc in nc.m.functions[0].allocations:
        if not isinstance(alloc, mybir.MemoryLocationSet):
            continue
        name = alloc.memorylocations[0].name
        if alloc.kind == "ExternalInput":
            if name != partition_name:
                in_names.append(name)
        elif alloc.kind == "ExternalOutput":
            out_names.append(name)
            shape = tuple(alloc.tensor_shape)
            dtype = mybir.dt.np(alloc.dtype)
            out_avals.append(jax.core.ShapedArray(shape, dtype))
            zero_outs.append(np.zeros(shape, dtype))
    n_params = len(in_names)
    all_names = in_names + out_names
    if partition_name is not None:
        all_names = all_names + [partition_name]

    def _body(*args):
        operands = list(args)
        if partition_name is not None:
            operands.append(bass2jax.partition_id_tensor())
        outs = bass2jax._bass_exec_p.bind(
            *operands,
            out_avals=tuple(out_avals),
            in_names=tuple(all_names),
            out_names=tuple(out_names),
            lowering_input_output_aliases=(),
            sim_require_finite=True,
            sim_require_nnan=True,
            nc=nc,
        )
        return tuple(outs)

    devices = jax.devices()[:NCORES]
    mesh = Mesh(np.asarray(devices), ("core",))
    sharded = jax.jit(
        shard_map(
            _body,
            mesh=mesh,
            in_specs=(PartitionSpec("core"),) * (n_params + len(out_avals)),
            out_specs=(PartitionSpec("core"),) * len(out_avals),
            check_rep=False,
        ),
        donate_argnums=tuple(range(n_params, n_params + len(out_avals))),
        keep_unused=True,
    )
    runner = {
        "jax": jax,
        "sharded": sharded,
        "in_names": in_names,
        "out_names": out_names,
        "out_avals": out_avals,
        "zero_outs": zero_outs,
        "n_params": n_params,
    }
    _cache["runner"] = runner
    return runner


def kernel(p, q, w1, w2, w3, w4, w5, w6, w7, w8, _time_iters=0):
    p = np.asarray(p, dtype=np.float32)
    q = np.asarray(q, dtype=np.float32)
    w_list = [np.asarray(w, dtype=np.float32) for w in (w1, w2, w3, w4, w5, w6, w7, w8)]

    r = _get_runner()
    jax = r["jax"]
    in_maps = [_prep_core_inputs(p, q, w_list, c) for c in range(NCORES)]
    concat_in = [
        np.concatenate([in_maps[c][name] for c in range(NCORES)], 0)
        for name in r["in_names"]
    ]
    concat_zeros = [
        np.zeros((NCORES * z.shape[0], *z.shape[1:]), z.dtype) for z in r["zero_outs"]
    ]
    out_arrs = r["sharded"](*concat_in, *concat_zeros)
    jax.block_until_ready(out_arrs)
    out = np.asarray(out_arrs[r["out_names"].index("out")])  # (64, 256, 160)

    if _time_iters:
        import time

        dev_in = [jax.device_put(a) for a in concat_in]
        jax.block_until_ready(dev_in)
        times = []
        for _ in range(_time_iters):
            zeros = [
                jax.device_put(np.zeros((NCORES * z.shape[0], *z.shape[1:]), z.dtype))
                for z in r["zero_outs"]
            ]
            jax.block_until_ready(zeros)
            t0 = time.perf_counter()
            o = r["sharded"](*dev_in, *zeros)
            jax.block_until_ready(o)
            times.append(time.perf_counter() - t0)
        kernel.last_exec_time_ns = int(min(times) * 1e9)
        kernel.all_times_ns = [int(t * 1e9) for t in times]
    return out


# revision 9
# speedup vs baseline: 1.5215x; 1.0719x over previous
"""BiMPM matching-layer kernel for Trainium2 (8 NeuronCores, pure data parallel).

v2: restructured for engine balance.
 - All matmuls bf16 except the m4-argmax chain (G, rrow, rrep) which must be
   fp32-exact so the argmax matches the reference (a flipped index gathers a
   completely different q0 row).
 - m2 (max-pooling): per perspective l, scale v2 by w2_l (rows) and 1/n2_l
   (cols) BEFORE the num matmul.  The [H,256] column-scale tile M_l =
   w2_l(h) * r2(l,j) is produced by one K=20 matmul against a host constant
   W2E whose l-th block is the one-hot row l scaled by w2_l.  The max over q
   runs hierarchically: Pool tensor_max halves PSUM (L1), DVE batched
   tensor_max (L2), DVE batched tensor_reduce tail.
Output (B, S1, 8*L), L=20. Batch 64 -> 8 per core.
"""
import numpy as np

B, S, H, L = 64, 256, 128, 20
NCORES = 8
BPC = B // NCORES
EPS = 1e-8

_cache = {}


def _build_bass():
    from contextlib import ExitStack

    import concourse.bass as bass
    import concourse.tile as tile
    from concourse import mybir

    f32 = mybir.dt.float32
    bf16 = mybir.dt.bfloat16
    AF = mybir.ActivationFunctionType
    OP = mybir.AluOpType
    AX = mybir.AxisListType

    nc = bass.Bass()

    # DRAM I/O (per core)
    pTb = nc.dram_tensor("pTb", [BPC, 2, H, S], bf16, kind="ExternalInput")
    qTb = nc.dram_tensor("qTb", [BPC, 2, H, S], bf16, kind="ExternalInput")
    pTf = nc.dram_tensor("pTf", [BPC, 2, H, S], f32, kind="ExternalInput")
    qTf = nc.dram_tensor("qTf", [BPC, 2, H, S], f32, kind="ExternalInput")
    qn = nc.dram_tensor("qn", [BPC, 2, S, H], f32, kind="ExternalInput")
    qnb = nc.dram_tensor("qnb", [BPC, 2, S, H], bf16, kind="ExternalInput")
    q0nb = nc.dram_tensor("q0nb", [2, S, H], bf16, kind="ExternalInput")
    w2T = nc.dram_tensor("w2T", [2, H, 81], f32, kind="ExternalInput")
    w2Tb = nc.dram_tensor("w2Tb", [2, H, 81], bf16, kind="ExternalInput")
    w2E = nc.dram_tensor("w2E", [2, L, L * H], bf16, kind="ExternalInput")
    iota2 = nc.dram_tensor("iota2", [H, 2], f32, kind="ExternalInput")
    onesr = nc.dram_tensor("onesr", [1, H], f32, kind="ExternalInput")
    onesrb = nc.dram_tensor("onesrb", [1, H], bf16, kind="ExternalInput")
    onesc = nc.dram_tensor("onesc", [H, 1], f32, kind="ExternalInput")
    ident = nc.dram_tensor("ident", [H, H], f32, kind="ExternalInput")
    out = nc.dram_tensor("out", [BPC, S, 8 * L], f32, kind="ExternalOutput")

    with tile.TileContext(nc) as tc, ExitStack() as ctx:
        cpool = ctx.enter_context(tc.tile_pool(name="consts", bufs=1))
        inp = ctx.enter_context(tc.tile_pool(name="inp", bufs=3))
        mid = ctx.enter_context(tc.tile_pool(name="mid", bufs=2))
        m2p = ctx.enter_context(tc.tile_pool(name="m2p", bufs=3))
        big = ctx.enter_context(tc.tile_pool(name="big", bufs=2))
        outp = ctx.enter_context(tc.tile_pool(name="outp", bufs=3))
        # PSUM (8 banks): each [H,512] f32 tile = 1 bank
        psQ_p = ctx.enter_context(tc.tile_pool(name="psQ", bufs=1, space="PSUM"))
        psG_p = ctx.enter_context(tc.tile_pool(name="psG", bufs=1, space="PSUM"))
        psWork_p = ctx.enter_context(tc.tile_pool(name="psWork", bufs=1, space="PSUM"))
        psSmall_p = ctx.enter_context(tc.tile_pool(name="psSmall", bufs=1, space="PSUM"))
        psM_p = ctx.enter_context(tc.tile_pool(name="psM", bufs=1, space="PSUM"))
        psNum_p = ctx.enter_context(tc.tile_pool(name="psNum", bufs=1, space="PSUM"))

        # ---- constants ----
        w2_sb = cpool.tile([H, 2, 81], f32)
        nc.sync.dma_start(out=w2_sb, in_=w2T[:].rearrange("d h c -> h d c"))
        w2b_sb = cpool.tile([H, 2, 81], bf16)
        nc.sync.dma_start(out=w2b_sb, in_=w2Tb[:].rearrange("d h c -> h d c"))
        w2e_sb = cpool.tile([L, 2, L * H], bf16)
        nc.sync.dma_start(out=w2e_sb, in_=w2E[:].rearrange("d l c -> l d c"))
        q0c = cpool.tile([H, 2, 2, H], bf16)
        nc.sync.dma_start(out=q0c, in_=q0nb[:].rearrange("d (c q) h -> q d c h", c=2))
        iota_sb = cpool.tile([H, 2], f32)
        nc.sync.dma_start(out=iota_sb, in_=iota2[:])
        iotab_sb = cpool.tile([H, 2], bf16)
        nc.vector.tensor_copy(iotab_sb, iota_sb)
        onesr_sb = cpool.tile([1, H], f32)
        nc.sync.dma_start(out=onesr_sb, in_=onesr[:])
        onesrb_sb = cpool.tile([1, H], bf16)
        nc.sync.dma_start(out=onesrb_sb, in_=onesrb[:])
        onesc_sb = cpool.tile([H, 1], f32)
        nc.sync.dma_start(out=onesc_sb, in_=onesc[:])
        ident_sb = cpool.tile([H, H], f32)
        nc.sync.dma_start(out=ident_sb, in_=ident[:])

        for b in range(BPC):
            outt0 = outp.tile([H, 8 * L], f32, name="outt0")
            outt1 = outp.tile([H, 8 * L], f32, name="outt1")
            outts = (outt0, outt1)
            for d in range(2):
                w2d = w2_sb[:, d, :]
                w2bd = w2b_sb[:, d, :]
                # ---- loads ----
                v1b = inp.tile([H, S], bf16, name="v1b")
                nc.sync.dma_start(out=v1b, in_=pTb[b, d])
                v2b = inp.tile([H, S], bf16, name="v2b")
                nc.sync.dma_start(out=v2b, in_=qTb[b, d])
                v1f = inp.tile([H, S], f32, name="v1f")
                nc.sync.dma_start(out=v1f, in_=pTf[b, d])
                v2f = inp.tile([H, S], f32, name="v2f")
                nc.sync.dma_start(out=v2f, in_=qTf[b, d])
                v2n = inp.tile([H, 2, H], f32, name="v2n")
                nc.sync.dma_start(out=v2n, in_=qn[b, d].rearrange("(c q) h -> q c h", c=2))
                v2nb = inp.tile([H, 2, H], bf16, name="v2nb")
                nc.sync.dma_start(out=v2nb, in_=qnb[b, d].rearrange("(c q) h -> q c h", c=2))

                # ---- squares (Pool: SBUF-only ops, keeps ACT free) ----
                v1sqb = mid.tile([H, S], bf16, name="v1sqb")
                nc.gpsimd.tensor_tensor(v1sqb, v1b, v1b, op=OP.mult)
                v2sqb = mid.tile([H, S], bf16, name="v2sqb")
                nc.gpsimd.tensor_tensor(v2sqb, v2b, v2b, op=OP.mult)
                v2sq = mid.tile([H, S], f32, name="v2sq")
                nc.scalar.square(v2sq, v2f)

                # ---- norms (one bank) ----
                # psQ: [rows 0:20 cols 0:256 n2m2T | row 32 cols 0:256 rrow |
                #       cols 256:418 n1 (2x81) | cols 418:420 ||v2_j||^2 fp32]
                psQ = psQ_p.tile([H, 512], f32, name="psQ")
                nc.tensor.matmul(psQ[0:L, 0:256], w2bd[:, 20:40], v2sqb, start=True, stop=True)
                nc.tensor.matmul(psQ[32:33, 0:256], onesc_sb, v2sq, start=True, stop=True)
                nc.tensor.matmul(psQ[:, 256:337], v1sqb[:, 0:H], w2bd, start=True, stop=True)
                nc.tensor.matmul(psQ[:, 337:418], v1sqb[:, H:S], w2bd, start=True, stop=True)
                nc.tensor.matmul(psQ[:, 418:419], v2sq[:, 0:H], onesc_sb, start=True, stop=True)
                nc.tensor.matmul(psQ[:, 419:420], v2sq[:, H:S], onesc_sb, start=True, stop=True)

                rs1a = mid.tile([H, 2 * 81], f32, name="rs1a")
                nc.vector.reciprocal(rs1a, psQ[:, 256:418])
                nc.scalar.sqrt(rs1a, rs1a)
                rs2y = mid.tile([H, 2], f32, name="rs2y")
                nc.vector.reciprocal(rs2y, psQ[:, 418:420])
                nc.scalar.sqrt(rs2y, rs2y)
                r2mf = mid.tile([L, 256], f32, name="r2mf")
                nc.vector.reciprocal(r2mf, psQ[0:L, 0:256])
                r2m = mid.tile([L, 256], bf16, name="r2m")
                nc.scalar.sqrt(r2m, r2mf)
                rrowf = mid.tile([1, 256], f32, name="rrowf")
                nc.vector.reciprocal(rrowf, psQ[32:33, 0:256])
                rrow = mid.tile([1, 256], f32, name="rrow")
                nc.scalar.sqrt(rrow, rrowf)

                # eps * ||v1|| (sign bias for m3)
                nv1e = mid.tile([H, 2], f32, name="nv1e")
                nc.scalar.sqrt(nv1e[:, 0:1], psQ[:, 336:337])
                nc.scalar.sqrt(nv1e[:, 1:2], psQ[:, 417:418])
                nc.scalar.mul(nv1e, nv1e, EPS)

                # ---- m1 ----
                tcolf = v2f[:, 255:256] if d == 0 else v2f[:, 0:1]
                sqt = mid.tile([H, 1], bf16, name="sqt")
                nc.scalar.square(sqt, tcolf)
                rhs1 = mid.tile([H, L], bf16, name="rhs1")
                nc.vector.tensor_scalar_mul(rhs1, w2bd[:, 0:20], tcolf)
                psSmall = psSmall_p.tile([H, 512], f32, name="psSmall")
                # psSmall: [c*100: num1 0:20 | num3 20:40 | num4 40:60 | n3 60:80 | n4 80:100]
                # [200:220 n2a-rep | 220:222 Gr | 222:242 (n2a row0) | 242:498 idxT row0]
                nc.tensor.matmul(psSmall[0:1, 222:242], sqt, w2bd[:, 0:20], start=True, stop=True)
                n2a_sb = mid.tile([1, L], bf16, name="n2a_sb")
                nc.scalar.copy(n2a_sb, psSmall[0:1, 222:242])
                nc.tensor.matmul(psSmall[:, 200:220], onesrb_sb, n2a_sb, start=True, stop=True)
                rs2rep = mid.tile([H, L], f32, name="rs2rep")
                nc.vector.reciprocal(rs2rep, psSmall[:, 200:220])
                nc.scalar.sqrt(rs2rep, rs2rep)
                nc.tensor.matmul(psSmall[:, 0:20], v1b[:, 0:H], rhs1, start=True, stop=True)
                nc.tensor.matmul(psSmall[:, 100:120], v1b[:, H:S], rhs1, start=True, stop=True)

                # ---- G (fp32, argmax) / GT (bf16, m3) ----
                # Pre-scale v2f columns by 1/||v2_j|| (replicated via stride-0 DMA) so
                # psG is the scaled-cos matrix directly -- no PSUM-x-PSUM TT needed.
                repG = mid.tile([H, 256], f32, name="repG")
                nc.sync.dma_start(out=repG, in_=rrow[0:1, :].unsqueeze(1).to_broadcast((1, H, 256)))
                v2fs = mid.tile([H, S], f32, name="v2fs")
                nc.gpsimd.tensor_tensor(v2fs, v2f, repG, op=OP.mult)
                psG = psG_p.tile([H, 512], f32, name="psG")
                nc.tensor.matmul(psG[:, 0:256], v1f[:, 0:H], v2fs, start=True, stop=True)
                nc.tensor.matmul(psG[:, 256:512], v1f[:, H:S], v2fs, start=True, stop=True)
                # GT lives in the psWork bank; GW/a4 overwrite it after GT_sb copies
                psWork = psWork_p.tile([H, 512], f32, name="psWork")
                nc.tensor.matmul(psWork[:, 0:256], v2b[:, 0:H], v1b, start=True, stop=True)
                nc.tensor.matmul(psWork[:, 256:512], v2b[:, H:S], v1b, start=True, stop=True)
                GT_sb = mid.tile([H, 2, S], bf16, name="GT_sb")
                nc.scalar.copy(GT_sb[:, 0, :], psWork[:, 0:256])
                nc.scalar.copy(GT_sb[:, 1, :], psWork[:, 256:512])

                # v2r rows (bf16) for GWT; y (fp32) for Gr sign
                v2rb = mid.tile([H, 2, H], bf16, name="v2rb")
                nc.vector.tensor_scalar_mul(v2rb[:, 0, :], v2nb[:, 0, :], rs2y[:, 0:1])
                nc.vector.tensor_scalar_mul(v2rb[:, 1, :], v2nb[:, 1, :], rs2y[:, 1:2])
                # GW at [0:256], att4T at [256:512]
                nc.tensor.matmul(psWork[:, 0:256], v2rb[:, 0, :], GT_sb[:, 0, :], start=True, stop=False)
                nc.tensor.matmul(psWork[:, 0:256], v2rb[:, 1, :], GT_sb[:, 1, :], start=False, stop=True)

                # y = sum_j v2_j / ||v2_j||  (all fp32: the m3 sign flips otherwise)
                psY = psSmall
                nc.tensor.matmul(psY[:, 220:221], v2n[:, 0, :], rs2y[:, 0:1], start=True, stop=False)
                nc.tensor.matmul(psY[:, 220:221], v2n[:, 1, :], rs2y[:, 1:2], start=False, stop=True)
                y_sb = mid.tile([H, 1], f32, name="y_sb")
                nc.scalar.copy(y_sb, psY[:, 220:221])
                nc.tensor.matmul(psY[:, 221:222], v1f[:, 0:H], y_sb, start=True, stop=True)
                nc.tensor.matmul(psY[:, 498:499], v1f[:, H:S], y_sb, start=True, stop=True)
                sgn0 = mid.tile([H, 1], f32, name="sgn0")
                nc.scalar.activation(sgn0, psY[:, 221:222], AF.Sign, bias=nv1e[:, 0:1], scale=1.0)
                sgn1 = mid.tile([H, 1], f32, name="sgn1")
                nc.scalar.activation(sgn1, psY[:, 498:499], AF.Sign, bias=nv1e[:, 1:2], scale=1.0)

                # ---- argmax head: top1 index of scaled G (all fp32) ----
                psM = psM_p.tile([H, 512], f32, name="psM")
                idxf = mid.tile([H, 2], f32, name="idxf")
                for c in range(2):
                    Gsc = mid.tile([H, 256], f32, name="Gsc")
                    nc.scalar.copy(Gsc, psG[:, 256 * c : 256 * c + 256])
                    top8 = mid.tile([H, 8], f32, name="top8")
                    nc.vector.max(top8, Gsc)
                    idx8 = mid.tile([H, 8], mybir.dt.uint32, name="idx8")
                    nc.vector.max_index(idx8, top8, Gsc)
                    nc.vector.tensor_copy(idxf[:, c : c + 1], idx8[:, 0:1])

                # m3 num/denominator matmuls (ready pre-loop).
                # ACT stages GW to SBUF bf16 once; Pool derives prod3 and sq3 from it.
                gwb = mid.tile([H, S], bf16, name="gwb")
                nc.scalar.copy(gwb, psWork[:, 0:256])
                prod3 = mid.tile([H, S], bf16, name="prod3")
                nc.gpsimd.tensor_tensor(prod3, v1b, gwb, op=OP.mult)
                sq3 = mid.tile([H, S], bf16, name="sq3")
                nc.gpsimd.tensor_tensor(sq3, gwb, gwb, op=OP.mult)
                for c in range(2):
                    base = 100 * c
                    sl = slice(c * H, c * H + H)
                    nc.tensor.matmul(psSmall[:, base + 20 : base + 40], prod3[:, sl], w2bd[:, 40:60], start=True, stop=True)
                    nc.tensor.matmul(psSmall[:, base + 60 : base + 80], sq3[:, sl], w2bd[:, 40:60], start=True, stop=True)

                # ---- m2 l-loop ----
                # Per 2 perspectives: M-pair (PE, K=20 one-hot matmul) -> psM bank,
                # DMA psM -> SBUF f32 (Pool may not touch PSUM), Pool v2s TT,
                # PE num matmuls, one strided DVE max drains each psNum to bf16.
                w2ed = w2e_sb[:, d, :]
                t2 = big.tile([H, L, 2], f32, name="t2")
                psNums = [
                    psNum_p.tile([H, 512], f32, name="psNumA"),
                    psNum_p.tile([H, 512], f32, name="psNumB"),
                ]
                psMs = [psM, psM_p.tile([H, 512], f32, name="psMB")]
                NS = L // 2
                mb_t = [None] * NS
                for s in range(NS + 2):
                    if s < NS:
                        psMx = psMs[s % 2]
                        # M for l=2s (cols 0:256) and l=2s+1 (cols 256:512)
                        nc.tensor.matmul(psMx[:, 0:256], w2ed[:, (2 * s) * H : (2 * s + 1) * H],
                                         r2m, start=True, stop=True)
                        nc.tensor.matmul(psMx[:, 256:512], w2ed[:, (2 * s + 1) * H : (2 * s + 2) * H],
                                         r2m, start=True, stop=True)
                        mb = m2p.tile([H, 512], bf16, name="mb")
                        mb_t[s] = mb
                        nc.scalar.copy(mb, psMx[:, 0:512])
                    if s >= 2:
                        t = s - 2
                        for j in range(2):
                            l = 2 * t + j
                            v2s = m2p.tile([H, S], bf16, name="v2s")
                            nc.gpsimd.tensor_tensor(v2s, mb_t[t][:, 256 * j : 256 * j + 256],
                                                    v2b, op=OP.mult)
                            psNum = psNums[l % 2]
                            nc.tensor.matmul(psNum[:, 0:256], v1b[:, 0:H], v2s, start=True, stop=True)
                            nc.tensor.matmul(psNum[:, 256:512], v1b[:, H:S], v2s, start=True, stop=True)
                            if l in (2, 5, 8, 12, 15, 18):
                                # drain via ACT copy; max tree on DVE at 2x from bf16 SBUF
                                stg = m2p.tile([H, 2, 2, H], bf16, name="stg")
                                nc.scalar.copy(stg, psNum[:].rearrange(
                                    "p (c two j) -> p c two j", c=2, two=2))
                                tm = m2p.tile([H, 2, H], bf16, name="tm")
                                nc.vector.tensor_max(tm, stg[:, :, 0, :], stg[:, :, 1, :])
                                nc.vector.tensor_reduce(t2[:, l, :], tm, axis=AX.X, op=OP.max)
                            else:
                                # drain psNum with one full max-reduce -> [H,2]
                                pv = psNum[:].rearrange("p (c j) -> p c j", c=2)
                                nc.vector.tensor_reduce(t2[:, l, :], pv, axis=AX.X, op=OP.max)

                # ---- post-loop: m4 index -> mask -> gather -> tails ----
                for c in range(2):
                    nc.tensor.transpose(psSmall[0:1, 242 + c * H : 242 + c * H + H],
                                        idxf[:, c : c + 1], ident_sb)
                idxT = mid.tile([1, 256], bf16, name="idxT")
                nc.scalar.copy(idxT, psSmall[0:1, 242:498])
                # replicate idx row via DMA; masks on Pool (all SBUF)
                idxrepS = mid.tile([H, 256], bf16, name="idxrepS")
                nc.sync.dma_start(out=idxrepS, in_=idxT[0:1, :].unsqueeze(1).to_broadcast((1, H, 256)))
                maskT0 = mid.tile([H, 256], bf16, name="maskT0")
                nc.vector.tensor_tensor(maskT0, idxrepS,
                                        iotab_sb[:, 0:1].to_broadcast((H, 256)), op=OP.is_equal)
                maskT1 = mid.tile([H, 256], bf16, name="maskT1")
                nc.vector.tensor_tensor(maskT1, idxrepS,
                                        iotab_sb[:, 1:2].to_broadcast((H, 256)), op=OP.is_equal)
                nc.tensor.matmul(psWork[:, 256:512], q0c[:, d, 0, :], maskT0, start=True, stop=False)
                nc.tensor.matmul(psWork[:, 256:512], q0c[:, d, 1, :], maskT1, start=False, stop=True)

                a4b = mid.tile([H, S], bf16, name="a4b")
                nc.scalar.copy(a4b, psWork[:, 256:512])
                prod4 = mid.tile([H, S], bf16, name="prod4")
                nc.gpsimd.tensor_tensor(prod4, v1b, a4b, op=OP.mult)
                sq4 = mid.tile([H, S], bf16, name="sq4")
                nc.gpsimd.tensor_tensor(sq4, a4b, a4b, op=OP.mult)
                for c in range(2):
                    base = 100 * c
                    sl = slice(c * H, c * H + H)
                    nc.tensor.matmul(psSmall[:, base + 40 : base + 60], prod4[:, sl], w2bd[:, 60:80], start=True, stop=True)
                    nc.tensor.matmul(psSmall[:, base + 80 : base + 100], sq4[:, sl], w2bd[:, 60:80], start=True, stop=True)

                # m2 finals (t2 is SBUF -> Pool)
                for c in range(2):
                    nc.gpsimd.tensor_tensor(outts[c][:, 40 + d * 20 : 40 + d * 20 + 20],
                                            t2[:, :, c], rs1a[:, 81 * c + 20 : 81 * c + 40], op=OP.mult)

                # m3 / m4 finals: one ACT copy of each psSmall chunk, Pool TTs after
                for c in range(2):
                    base = 100 * c
                    smf = mid.tile([H, 100], f32, name="smf")
                    nc.scalar.copy(smf, psSmall[:, base : base + 100])
                    rsq34 = mid.tile([H, 40], f32, name="rsq34")
                    nc.vector.reciprocal(rsq34, psSmall[:, base + 60 : base + 100])
                    nc.scalar.sqrt(rsq34, rsq34)
                    sgn = sgn0 if c == 0 else sgn1
                    t1 = mid.tile([H, L], f32, name="t1x")
                    nc.gpsimd.tensor_tensor(t1, smf[:, 0:20], rs1a[:, 81 * c : 81 * c + 20], op=OP.mult)
                    nc.gpsimd.tensor_tensor(outts[c][:, d * 20 : d * 20 + 20], t1, rs2rep, op=OP.mult)
                    t3 = mid.tile([H, L], f32, name="t3")
                    nc.gpsimd.tensor_tensor(t3, smf[:, 20:40],
                                            rs1a[:, 81 * c + 40 : 81 * c + 60], op=OP.mult)
                    t3b = mid.tile([H, L], f32, name="t3b")
                    nc.gpsimd.tensor_tensor(t3b, t3, rsq34[:, 0:20], op=OP.mult)
                    nc.scalar.mul(outts[c][:, 80 + d * 20 : 80 + d * 20 + 20], t3b, sgn)
                    t4 = mid.tile([H, L], f32, name="t4")
                    nc.gpsimd.tensor_tensor(t4, smf[:, 40:60],
                                            rs1a[:, 81 * c + 60 : 81 * c + 80], op=OP.mult)
                    nc.gpsimd.tensor_tensor(outts[c][:, 120 + d * 20 : 120 + d * 20 + 20],
                                            t4, rsq34[:, 20:40], op=OP.mult)

            # ---- store ----
            nc.sync.dma_start(out=out[b, 0:H, :], in_=outt0)
            nc.sync.dma_start(out=out[b, H:S, :], in_=outt1)

    return nc


def _prep_core_inputs(p, q, w_list, core):
    """Host-side layout prep for one core. Layout transforms + weight-only math."""
    import ml_dtypes

    bf16 = ml_dtypes.bfloat16
    sl = slice(core * BPC, (core + 1) * BPC)
    p8 = np.ascontiguousarray(p[sl])  # (BPC, 256, 256)
    q8 = np.ascontiguousarray(q[sl])
    pT = np.ascontiguousarray(p8.reshape(BPC, S, 2, H).transpose(0, 2, 3, 1))
    qT = np.ascontiguousarray(q8.reshape(BPC, S, 2, H).transpose(0, 2, 3, 1))
    qn = np.ascontiguousarray(q8.reshape(BPC, S, 2, H).transpose(0, 2, 1, 3))
    q0n = np.ascontiguousarray(q[0].reshape(S, 2, H).transpose(1, 0, 2))  # (2, S, H)

    w2T = np.empty((2, H, 81), np.float32)
    w2E = np.zeros((2, L, L * H), np.float32)
    for d in range(2):
        ws = w_list[d::2]  # fw: w1,w3,w5,w7 ; bw: w2,w4,w6,w8
        cat = np.concatenate([w * w for w in ws] + [np.ones((1, H), np.float32)], 0)
        w2T[d] = cat.T
        w2m2 = ws[1] * ws[1]  # (L, H) m2 weights squared
        for l in range(L):
            w2E[d, l, l * H : (l + 1) * H] = w2m2[l]
    iota2 = np.stack([np.arange(H, dtype=np.float32), np.arange(H, 2 * H, dtype=np.float32)], 1)

    return {
        "pTb": pT.astype(bf16),
        "qTb": qT.astype(bf16),
        "pTf": pT,
        "qTf": qT,
        "qn": qn,
        "qnb": qn.astype(bf16),
        "q0nb": q0n.astype(bf16),
        "w2T": w2T,
        "w2Tb": w2T.astype(bf16),
        "w2E": w2E.astype(bf16),
        "iota2": np.ascontiguousarray(iota2),
        "onesr": np.ones((1, H), np.float32),
        "onesrb": np.ones((1, H), bf16),
        "onesc": np.ones((H, 1), np.float32),
        "ident": np.eye(H, dtype=np.float32),
    }


def _legalize_bir(bir_bytes):
    """Move all but one sync-wait of each instruction onto inserted same-engine Drains
    (this neuronxcc build rejects >1 sync wait per instruction)."""
    import json as _json

    d = _json.loads(bir_bytes)
    n = 0
    for fnd in d["functions"]:
        for blk in fnd["blocks"]:
            insts = blk.get("instructions") or []
            out = []
            for ins in insts:
                si = ins.get("sync_info") or {}
                w = si.get("on_wait") or []
                if len(w) > 1:
                    for extra in w[:-1]:
                        out.append(
                            {
                                "debug": ins.get("debug", 0),
                                "engine": ins.get("engine"),
                                "ins": [],
                                "outs": [],
                                "is_reset_sema": False,
                                "name": f"I-legalw-{n}",
                                "opcode": "Drain",
                                "sync_info": {"on_update": [], "on_wait": [extra]},
                            }
                        )
                        n += 1
                    si["on_wait"] = [w[-1]]
                out.append(ins)
            blk["instructions"] = out
    return _json.dumps(d).encode(), n


def _install_legalizer():
    if _cache.get("legalizer"):
        return
    from concourse import bass2jax, bass_utils

    orig = bass_utils.compile_bir_kernel

    def patched(bir_json, tmpdir, neff_name="file.neff"):
        fixed, n = _legalize_bir(bir_json)
        return orig(fixed, tmpdir, neff_name)

    bass2jax.compile_bir_kernel = patched
    _cache["legalizer"] = True


def _get_runner():
    if "runner" in _cache:
        return _cache["runner"]

    import jax
    from jax.sharding import Mesh, PartitionSpec
    from jax.experimental.shard_map import shard_map

    import concourse.mybir as mybir
    from concourse import bass2jax

    if "nc" not in _cache:
        _cache["nc"] = _build_bass()
    nc = _cache["nc"]

    bass2jax.install_neuronx_cc_hook()
    _install_legalizer()
    assert nc.dbg_addr is None
    partition_name = nc.partition_id_tensor.name if nc.partition_id_tensor else None

    in_names, out_names, out_avals, zero_outs = [], [], [], []
    for alloc in nc.m.functions[0].allocations:
        if not isinstance(alloc, mybir.MemoryLocationSet):
            continue
        name = alloc.memorylocations[0].name
        if alloc.kind == "ExternalInput":
            if name != partition_name:
                in_names.append(name)
        elif alloc.kind == "ExternalOutput":
            out_names.append(name)
            shape = tuple(alloc.tensor_shape)
            dtype = mybir.dt.np(alloc.dtype)
            out_avals.append(jax.core.ShapedArray(shape, dtype))
            zero_outs.append(np.zeros(shape, dtype))
    n_params = len(in_names)
    all_names = in_names + out_names
    if partition_name is not None:
        all_names = all_names + [partition_name]

    def _body(*args):
        operands = list(args)
        if partition_name is not None:
            operands.append(bass2jax.partition_id_tensor())
        outs = bass2jax._bass_exec_p.bind(
            *operands,
            out_avals=tuple(out_avals),
            in_names=tuple(all_names),
            out_names=tuple(out_names),
            lowering_input_output_aliases=(),
            sim_require_finite=True,
            sim_require_nnan=True,
            nc=nc,
        )
        return tuple(outs)

    devices = jax.devices()[:NCORES]
    mesh = Mesh(np.asarray(devices), ("core",))
    sharded = jax.jit(
        shard_map(
            _body,
            mesh=mesh,
            in_specs=(PartitionSpec("core"),) * (n_params + len(out_avals)),
            out_specs=(PartitionSpec("core"),) * len(out_avals),
            check_rep=False,
        ),
        donate_argnums=tuple(range(n_params, n_params + len(out_avals))),
        keep_unused=True,
    )
    runner = {
        "jax": jax,
        "sharded": sharded,
        "in_names": in_names,
        "out_names": out_names,
        "out_avals": out_avals,
        "zero_outs": zero_outs,
        "n_params": n_params,
    }
    _cache["runner"] = runner
    return runner


def kernel(p, q, w1, w2, w3, w4, w5, w6, w7, w8, _time_iters=0):
    p = np.asarray(p, dtype=np.float32)
    q = np.asarray(q, dtype=np.float32)
    w_list = [np.asarray(w, dtype=np.float32) for w in (w1, w2, w3, w4, w5, w6, w7, w8)]

    r = _get_runner()
    jax = r["jax"]
    in_maps = [_prep_core_inputs(p, q, w_list, c) for c in range(NCORES)]
    concat_in = [
        np.concatenate([in_maps[c][name] for c in range(NCORES)], 0)
        for name in r["in_names"]
    ]
    concat_zeros = [
        np.zeros((NCORES * z.shape[0], *z.shape[1:]), z.dtype) for z in r["zero_outs"]
    ]
    out_arrs = r["sharded"](*concat_in, *concat_zeros)
    jax.block_until_ready(out_arrs)
    out = np.asarray(out_arrs[r["out_names"].index("out")])  # (64, 256, 160)

    if _time_iters:
        import time

        dev_in = [jax.device_put(a) for a in concat_in]
        jax.block_until_ready(dev_in)
        times = []
        for _ in range(_time_iters):
            zeros = [
                jax.device_put(np.zeros((NCORES * z.shape[0], *z.shape[1:]), z.dtype))
                for z in r["zero_outs"]
            ]
            jax.block_until_ready(zeros)
            t0 = time.perf_counter()
            o = r["sharded"](*dev_in, *zeros)
            jax.block_until_ready(o)
            times.append(time.perf_counter() - t0)
        kernel.last_exec_time_ns = int(min(times) * 1e9)
        kernel.all_times_ns = [int(t * 1e9) for t in times]
    return out
